# revision 6
# baseline (speedup 1.0000x reference)
"""Trainium2 Bass kernel for a 2-layer shared-weight LSTM with residual.

Problem: x:[1024,200,128], W/U:[128,512], b:[512]; two stacked LSTM layers
sharing (W,U,b); layer 2 has a residual connection; seq_len is ignored by the
reference (full T steps).

Sharding: data-parallel over batch: 1024 = 8 cores x 128 rows.

Device layout: gate features on SBUF partitions, batch on the free axis.
Host pre-transposes x to [T, D, B_local] (bf16).

Key restructurings vs the straightforward schedule:

* All-tanh transform: sigma(z) = (tanh(z/2)+1)/2, so the f/i/o weight chunks
  are pre-scaled by 1/2 and EVERY gate activation is a Tanh.  The (tau+1)
  factors fold into scalar_tensor_tensor ops at no extra cost by carrying the
  recurrent state as H = 2h and S = 2c (exact bf16 exponent shifts).  This
  lets the i and g gate activations merge into ONE 512-wide ScalarE act.
* Layer-2 phase delay d=2: unit v computes L1 step v and L2 step v-2.  All
  L2 matmul inputs except U*h2raw are then old enough to issue early, so only
  TWO late matmuls per gate chunk sit between h1-ready and that chunk's
  activation.  The residual sum Y2 = H1 + H2RAW only feeds the y-output DMA
  (computed on GpSimd, fully off the recurrence).
* Bias enters via 1-row matmul "openers" for the merged i,g chunks (act bias
  is per-partition and cannot differ across merged chunks); f,o use act bias.
* psum layout: one [128, 1024] f32 tile per unit, chunk order [i,g,f,o],
  each chunk [L2 half | L1 half]; i,g adjacent so tanh(i,g) is one act.

Recurrence per unit (steady state), with tau* = tanh gate outputs:
  B1 = (tau_f + 1) * S1_prev          # = 4*sig(f)*c1_prev   (stt)
  A1 = (tau_i + 1) * tau_g            # = 2*sig(i)*tanh(g)   (stt)
  S1 = 0.5*B1 + A1                    # = 2*c1               (stt)
  tc1 = tanh(0.5 * S1)                # ScalarE act, scale=0.5
  H1 = (tau_o + 1) * tc1              # = 2*h1               (stt)
Layer 2 identical with its own S2/H2RAW; y = (H1 + H2RAW)/2 with the /2
applied on the host after gather.
"""

import numpy as np
import ml_dtypes

import concourse.bass as bass
import concourse.tile as tile
from concourse import bacc, mybir
from concourse.bass_utils import run_bass_kernel_spmd

B, T, D = 1024, 200, 128
NCORES = 8
BL = B // NCORES  # 128 batch rows per core
NU = T + 2        # units: 0..T+1 (L1 at v<=T-1, L2 step v-2 at v>=2)

F32 = mybir.dt.float32
BF16 = mybir.dt.bfloat16

# gate order in W/U/b: i, f, g, o  (Keras LSTMCell)
GI, GF, GG, GO = 0, 1, 2, 3
ALPHA = {GI: 0.5, GF: 0.5, GG: 1.0, GO: 0.5}
# psum chunk column base: [i, g, f, o] so (i,g) merge into one act
PBASE = {GI: 0, GG: 2 * BL, GF: 4 * BL, GO: 6 * BL}
CHUNKS_LATE = (GF, GI, GG, GO)  # f first: B1 = (tau_f+1)*S1_prev earliest

import os
N_JUNK = int(os.environ.get("K_JUNK", "0"))


def _build(nc):
    x_d = nc.dram_tensor("x", [T, D, BL], BF16, kind="ExternalInput")
    wa_d = nc.dram_tensor("wa", [D, 4 * D], BF16, kind="ExternalInput")
    wc_d = nc.dram_tensor("wc", [D, 4 * D], BF16, kind="ExternalInput")
    ub_d = nc.dram_tensor("ub", [D, 4 * D], BF16, kind="ExternalInput")
    b_d = nc.dram_tensor("bias", [D, 4], F32, kind="ExternalInput")
    bc_d = nc.dram_tensor("bcst", [1, 2 * BL], BF16, kind="ExternalInput")
    y_d = nc.dram_tensor("y", [T, D, BL], BF16, kind="ExternalOutput")

    TANH = mybir.ActivationFunctionType.Tanh
    ADD = mybir.AluOpType.add
    MUL = mybir.AluOpType.mult

    def l2s(k):  # L2 half of chunk k in psum/gates
        return slice(PBASE[k], PBASE[k] + BL)

    def l1s(k):  # L1 half
        return slice(PBASE[k] + BL, PBASE[k] + 2 * BL)

    def chs(k):  # whole chunk
        return slice(PBASE[k], PBASE[k] + 2 * BL)

    with tile.TileContext(nc) as tc:
        with (
            tc.tile_pool(name="singles", bufs=1) as singles,
            tc.tile_pool(name="xbuf", bufs=4) as xpool,
            tc.tile_pool(name="hbuf", bufs=5) as hpool,
            tc.tile_pool(name="hrbuf", bufs=3) as hrpool,
            tc.tile_pool(name="sbuf1", bufs=2) as s1pool,
            tc.tile_pool(name="sbuf2", bufs=2) as s2pool,
            tc.tile_pool(name="psum", bufs=2, space="PSUM") as pspool,
            tc.tile_pool(name="gates", bufs=2) as gpool,
            tc.tile_pool(name="tmp", bufs=3) as tpool,
            tc.tile_pool(name="yst", bufs=4) as ypool,
        ):
            wa_sb = singles.tile([D, 4 * D], BF16)
            wc_sb = singles.tile([D, 4 * D], BF16)
            ub_sb = singles.tile([D, 4 * D], BF16)
            b_sb = singles.tile([D, 4], F32)
            bc_sb = singles.tile([1, 2 * BL], BF16)
            ones_sb = singles.tile([1, 2 * BL], BF16)
            nc.sync.dma_start(wa_sb[:], wa_d[:])
            nc.sync.dma_start(wc_sb[:], wc_d[:])
            nc.sync.dma_start(ub_sb[:], ub_d[:])
            nc.sync.dma_start(b_sb[:], b_d[:])
            nc.sync.dma_start(bc_sb[:], bc_d[:])
            nc.vector.memset(ones_sb[:], 1.0)

            def wak(k):
                return wa_sb[:, k * D:(k + 1) * D]

            def wck(k):
                return wc_sb[:, k * D:(k + 1) * D]

            def ubk(k):
                return ub_sb[:, k * D:(k + 1) * D]

            def bk(k):
                return b_sb[:, k:k + 1]

            xt = {}    # x(v) tiles
            h1 = {}    # H1(v) = 2*h1(v)
            hr = {}    # HR(s) = 2*h2raw(s)
            s1 = {}    # S1(v) = 2*c1(v)
            s2 = {}    # S2(s) = 2*c2(s)

            # prefetch x(0), x(1)
            for v in (0, 1):
                xt[v] = xpool.tile([D, BL], BF16, tag="x", name=f"x{v}")
                nc.sync.dma_start(xt[v][:], x_d[v])

            prev_late_f = None

            for v in range(NU):
                has_l1 = v <= T - 1
                has_l1_late = 1 <= v <= T - 1
                has_l2 = v >= 2
                has_l2_ub = v >= 3
                s = v - 2  # L2 step index

                # prefetch x(v+2)
                if v + 2 <= T - 1:
                    xt[v + 2] = xpool.tile([D, BL], BF16, tag="x",
                                           name=f"x{v+2}")
                    nc.sync.dma_start(xt[v + 2][:], x_d[v + 2])

                # psum: one accumulation bank per tile; EXACTLY one
                # start=True per bank (it clears has_written for the WHOLE
                # bank), executing first; stop=True only on the last writer.
                zig = pspool.tile([D, 4 * BL], F32, tag="zig", name=f"zig{v}")
                zf = pspool.tile([D, 2 * BL], F32, tag="zf", name=f"zf{v}")
                zo = pspool.tile([D, 2 * BL], F32, tag="zo", name=f"zo{v}")
                g = gpool.tile([D, 8 * BL], BF16, tag="g", name=f"g{v}")

                # zig cols: [i-L2 | i-L1 | g-L2 | g-L1]; zf/zo: [L2 | L1]
                def zt(k):
                    return {GI: (zig, 0), GG: (zig, 2 * BL),
                            GF: (zf, 0), GO: (zo, 0)}[k]

                def zl1(k):
                    t, off = zt(k)
                    return t[:, off + BL:off + 2 * BL]

                def zl2(k):
                    t, off = zt(k)
                    return t[:, off:off + BL]

                def zch(k):
                    t, off = zt(k)
                    return t[:, off:off + 2 * BL]

                if has_l1 and has_l2:
                    bw = slice(0, 2 * BL)
                elif has_l1:
                    bw = slice(BL, 2 * BL)
                else:
                    bw = slice(0, BL)

                def bank_of(k):
                    return {GI: "ig", GG: "ig", GF: "f", GO: "o"}[k]

                # planned writer args per bank, in required execution order
                plan = {"ig": [], "f": [], "o": []}
                # bias openers for i,g (first in the shared ig bank)
                for k in (GI, GG):
                    t, off = zt(k)
                    col = slice(off + bw.start, off + bw.stop)
                    bcol = slice(0, BL) if k == GI else slice(BL, 2 * BL)
                    plan[bank_of(k)].append((t[:, col], bc_sb[:, bcol],
                                             ones_sb[:, bw]))
                if has_l1:
                    for k in (GF, GI, GG, GO):
                        plan[bank_of(k)].append((zl1(k), wak(k), xt[v][:]))
                if has_l2:
                    for k in (GF, GI, GG, GO):
                        plan[bank_of(k)].append((zl2(k), wck(k), h1[s][:]))
                if has_l2_ub:
                    for k in (GF, GI, GG, GO):
                        plan[bank_of(k)].append((zl2(k), ubk(k),
                                                 h1[s - 1][:]))
                n_early = {b: len(plan[b]) for b in plan}
                lates = {k: [] for k in (GI, GF, GG, GO)}
                for k in (GF, GI, GG, GO):
                    if has_l1_late:
                        lates[k].append((zl1(k), ubk(k), h1[v - 1][:]))
                    if has_l2_ub:
                        lates[k].append((zl2(k), ubk(k), hr[s - 1][:]))
                    plan[bank_of(k)].extend(lates[k])
                # ig bank: i's lates come before g's lates (ladder order)

                nemit = {b: 0 for b in plan}
                last_mm = {b: None for b in plan}

                def emit(b):
                    i = nemit[b]
                    out, lhsT, rhs = plan[b][i]
                    m = nc.tensor.matmul(out, lhsT, rhs,
                                         start=(i == 0),
                                         stop=(i == len(plan[b]) - 1))
                    if last_mm[b] is not None:
                        tile.add_dep_helper(m.ins, last_mm[b].ins,
                                            sync=False,
                                            reason="psum bank order")
                    nemit[b] += 1
                    last_mm[b] = m
                    return m

                # emit earlies round-robin (f, ig, o) so f finishes first
                earlies = []
                cnt = dict(n_early)
                while any(nemit[b] < n_early[b] for b in plan):
                    for b in ("f", "ig", "o"):
                        if nemit[b] < n_early[b]:
                            earlies.append(emit(b))
                if prev_late_f is not None:
                    for m in earlies:
                        tile.add_dep_helper(m.ins, prev_late_f.ins,
                                            sync=False,
                                            reason="early after prev late-f")

                # ---- late matmuls + gate activation ladder ----
                def act_cols(k):
                    if has_l1 and has_l2:
                        return chs(k)
                    return l1s(k) if has_l1 else l2s(k)

                def zact_cols(k):
                    if has_l1 and has_l2:
                        return zch(k)
                    return zl1(k) if has_l1 else zl2(k)

                first_late = None
                for k in CHUNKS_LATE:
                    b = bank_of(k)
                    for _ in lates[k]:
                        m = emit(b)
                        if first_late is None:
                            first_late = m
                    if k == GF:
                        nc.scalar.activation(g[:, act_cols(GF)],
                                             zact_cols(GF),
                                             TANH, bias=bk(GF))
                    elif k == GG:
                        if has_l1 and has_l2:
                            nc.scalar.activation(g[:, 0:4 * BL],
                                                 zig[:, 0:4 * BL], TANH)
                        else:
                            nc.scalar.activation(g[:, act_cols(GI)],
                                                 zact_cols(GI), TANH)
                            nc.scalar.activation(g[:, act_cols(GG)],
                                                 zact_cols(GG), TANH)
                    elif k == GO:
                        nc.scalar.activation(g[:, act_cols(GO)],
                                             zact_cols(GO),
                                             TANH, bias=bk(GO))
                if first_late is not None:
                    prev_late_f = first_late

                # ---- pointwise tails ----
                tcb = tpool.tile([D, 2 * BL], BF16, tag="tc", name=f"tc{v}")

                if has_l1:
                    ns1 = s1pool.tile([D, BL], BF16, tag="s1", name=f"s1_{v}")
                    if has_l1_late:
                        b1 = tpool.tile([D, BL], BF16, tag="b1",
                                        name=f"b1_{v}")
                        a1 = tpool.tile([D, BL], BF16, tag="a1",
                                        name=f"a1_{v}")
                        nc.vector.scalar_tensor_tensor(
                            b1[:], g[:, l1s(GF)], 1.0, s1[v - 1][:],
                            ADD, MUL)
                        nc.vector.scalar_tensor_tensor(
                            a1[:], g[:, l1s(GI)], 1.0, g[:, l1s(GG)],
                            ADD, MUL)
                        nc.vector.scalar_tensor_tensor(
                            ns1[:], b1[:], 0.5, a1[:], MUL, ADD)
                    else:
                        nc.vector.scalar_tensor_tensor(
                            ns1[:], g[:, l1s(GI)], 1.0, g[:, l1s(GG)],
                            ADD, MUL)
                    s1[v] = ns1

                if has_l2:
                    ns2 = s2pool.tile([D, BL], BF16, tag="s2", name=f"s2_{v}")
                    if has_l2_ub:
                        b2 = tpool.tile([D, BL], BF16, tag="b2",
                                        name=f"b2_{v}")
                        a2 = tpool.tile([D, BL], BF16, tag="a2",
                                        name=f"a2_{v}")
                        nc.vector.scalar_tensor_tensor(
                            b2[:], g[:, l2s(GF)], 1.0, s2[v - 1][:],
                            ADD, MUL)
                        nc.vector.scalar_tensor_tensor(
                            a2[:], g[:, l2s(GI)], 1.0, g[:, l2s(GG)],
                            ADD, MUL)
                        nc.vector.scalar_tensor_tensor(
                            ns2[:], b2[:], 0.5, a2[:], MUL, ADD)
                    else:
                        nc.vector.scalar_tensor_tensor(
                            ns2[:], g[:, l2s(GI)], 1.0, g[:, l2s(GG)],
                            ADD, MUL)
                    s2[v] = ns2

                # tanh(c) acts (ScalarE), then H muls (DVE)
                if has_l1:
                    nc.scalar.activation(tcb[:, 0:BL], s1[v][:], TANH,
                                         scale=0.5)
                if has_l2:
                    nc.scalar.activation(tcb[:, BL:2 * BL], s2[v][:], TANH,
                                         scale=0.5)
                if has_l1:
                    nh1 = hpool.tile([D, BL], BF16, tag="h1", name=f"h1_{v}")
                    nc.vector.scalar_tensor_tensor(
                        nh1[:], g[:, l1s(GO)], 1.0, tcb[:, 0:BL], ADD, MUL)
                    h1[v] = nh1
                if has_l2:
                    nhr = hrpool.tile([D, BL], BF16, tag="hr", name=f"hr{v}")
                    nc.vector.scalar_tensor_tensor(
                        nhr[:], g[:, l2s(GO)], 1.0, tcb[:, BL:2 * BL],
                        ADD, MUL)
                    hr[s] = nhr
                    # residual -> y2(s) = H1(s) + HR(s); host halves it
                    yt = ypool.tile([D, BL], BF16, tag="yst", name=f"y{s}")
                    nc.gpsimd.tensor_add(yt[:], h1[s][:], nhr[:])
                    nc.sync.dma_start(y_d[s], yt[:])

                # drop old ring entries
                xt.pop(v, None)
                h1.pop(v - 4, None)
                hr.pop(s - 2, None)
                s1.pop(v - 1, None)
                s2.pop(v - 1, None)

    nc.finalize()
    return nc


_CACHED = {}


def _get_nc():
    if "nc" not in _CACHED:
        nc = bacc.Bacc("TRN2", target_bir_lowering=False, debug=False,
                       num_devices=NCORES)
        _CACHED["nc"] = _build(nc)
    return _CACHED["nc"]


def kernel(x, W, U, b, seq_len):
    assert x.shape == (B, T, D)
    nc = _get_nc()

    bf = ml_dtypes.bfloat16
    Wf = np.asarray(W, dtype=np.float32)
    Uf = np.asarray(U, dtype=np.float32)
    bfv = np.asarray(b, dtype=np.float32)

    alpha = np.empty(4 * D, dtype=np.float32)
    for k, a in ALPHA.items():
        alpha[k * D:(k + 1) * D] = a

    WA = np.ascontiguousarray((Wf * alpha[None, :]).astype(bf))
    WC = np.ascontiguousarray((Wf * (0.5 * alpha)[None, :]).astype(bf))
    UB = np.ascontiguousarray((Uf * (0.5 * alpha)[None, :]).astype(bf))
    b_half = bfv * alpha
    bsb = np.ascontiguousarray(b_half.reshape(4, D).T)  # [D, 4] f32
    bcst = np.ascontiguousarray(
        np.concatenate([b_half[GI * D:(GI + 1) * D],
                        b_half[GG * D:(GG + 1) * D]])[None, :].astype(bf))

    in_maps = []
    for c in range(NCORES):
        xc = np.ascontiguousarray(
            np.asarray(x[c * BL:(c + 1) * BL], dtype=np.float32)
            .transpose(1, 2, 0).astype(bf))  # [T, D, BL] bf16
        in_maps.append({"x": xc, "wa": WA, "wc": WC, "ub": UB,
                        "bias": bsb, "bcst": bcst})

    res = run_bass_kernel_spmd(nc, in_maps, core_ids=list(range(NCORES)))

    y = np.empty((B, T, D), dtype=np.float32)
    for c in range(NCORES):
        # y_T [T, D, BL] bf16 (holds 2*y) -> [BL, T, D] fp32, halved
        y[c * BL:(c + 1) * BL] = (
            res.results[c]["y"].astype(np.float32).transpose(2, 0, 1)) * 0.5
    return y


# revision 7
# speedup vs baseline: 1.1607x; 1.1607x over previous
"""Trainium2 Bass kernel for a 2-layer shared-weight LSTM with residual.

Problem: x:[1024,200,128], W/U:[128,512], b:[512]; two stacked LSTM layers
sharing (W,U,b); layer 2 has a residual connection; seq_len is ignored by the
reference (full T steps).

Sharding: data-parallel over batch: 1024 = 8 cores x 128 rows.

Device layout: gate features on SBUF partitions, batch on the free axis.
Host pre-transposes x to [T, D, B_local] (bf16).

Schedule: layer-2 phase delay d=2 — unit v computes L1 step v and L2 step
v-2.  With the residual split (U*h2n = U*h1 + U*h2raw), every L2 matmul
input except U*h2raw(v-3) is at least two units old, so per gate chunk only
TWO late matmuls (U*h1(v-1) -> L1 half, U*h2raw(v-3) -> L2 half) sit between
h1-ready and that chunk's activation.  Earlies of unit v+1 are dep-pinned
after unit v's LAST late so the scheduler cannot wedge them into the ladder.

psum: one [128,256] tile (= one full psum bank after padding) per gate
chunk, exactly one start=True opener per bank executing first (start=True
clears has_written for the WHOLE bank), stop=True on the final writer.

The pointwise tail is plain tensor_tensor ops (DVE 2x mode, ~136ns chain
pitch): m1=sig(f)*c, m2=sig(i)*tanh(g), c'=m1+m2, tanh(c'), h=sig(o)*tc.
y(s) = h1(s) + h2raw(s) on GpSimd, off the recurrence entirely.
"""

import numpy as np
import ml_dtypes

import concourse.bass as bass
import concourse.tile as tile
from concourse import bacc, mybir
from concourse.bass_utils import run_bass_kernel_spmd

B, T, D = 1024, 200, 128
NCORES = 8
BL = B // NCORES  # 128 batch rows per core
NU = T + 2        # units: 0..T+1 (L1 at v<=T-1, L2 step v-2 at v>=2)

F32 = mybir.dt.float32
BF16 = mybir.dt.bfloat16

# gate order in W/U/b: i, f, g, o  (Keras LSTMCell)
GI, GF, GG, GO = 0, 1, 2, 3
CHUNKS = (GF, GI, GG, GO)  # ladder order: f first (m1 = sig(f)*c earliest)


def _build(nc):
    x_d = nc.dram_tensor("x", [T, D, BL], BF16, kind="ExternalInput")
    w_d = nc.dram_tensor("w", [D, 4 * D], BF16, kind="ExternalInput")
    u_d = nc.dram_tensor("u", [D, 4 * D], BF16, kind="ExternalInput")
    b_d = nc.dram_tensor("bias", [D, 4], F32, kind="ExternalInput")
    y_d = nc.dram_tensor("y", [T, D, BL], BF16, kind="ExternalOutput")

    SIG = mybir.ActivationFunctionType.Sigmoid
    TANH = mybir.ActivationFunctionType.Tanh
    FUNC = {GF: SIG, GI: SIG, GG: TANH, GO: SIG}

    L2 = slice(0, BL)        # L2 half of each chunk (psum + gate tiles)
    L1 = slice(BL, 2 * BL)   # L1 half

    with tile.TileContext(nc) as tc:
        with (
            tc.tile_pool(name="singles", bufs=1) as singles,
            tc.tile_pool(name="xbuf", bufs=4) as xpool,
            tc.tile_pool(name="hbuf", bufs=5) as hpool,
            tc.tile_pool(name="hrbuf", bufs=3) as hrpool,
            tc.tile_pool(name="psum", bufs=2, space="PSUM") as pspool,
            tc.tile_pool(name="gates", bufs=2) as gpool,
            tc.tile_pool(name="tmp", bufs=3) as tpool,
            tc.tile_pool(name="yst", bufs=4) as ypool,
        ):
            w_sb = singles.tile([D, 4 * D], BF16)
            u_sb = singles.tile([D, 4 * D], BF16)
            b_sb = singles.tile([D, 4], F32)
            nc.sync.dma_start(w_sb[:], w_d[:])
            nc.sync.dma_start(u_sb[:], u_d[:])
            nc.sync.dma_start(b_sb[:], b_d[:])

            # persistent cell state: cols 0:BL = c2, BL:2BL = c1 (bf16)
            c_both = singles.tile([D, 2 * BL], BF16)
            nc.vector.memset(c_both[:], 0.0)

            def wk(k):
                return w_sb[:, k * D:(k + 1) * D]

            def uk(k):
                return u_sb[:, k * D:(k + 1) * D]

            def bk(k):
                return b_sb[:, k:k + 1]

            xt = {}   # x(v)
            h1 = {}   # h1(v)
            hr = {}   # h2raw(s)

            for v in (0, 1):
                xt[v] = xpool.tile([D, BL], BF16, tag="x", name=f"x{v}")
                nc.sync.dma_start(xt[v][:], x_d[v])

            prev_last_late = None

            for v in range(NU):
                has_l1 = v <= T - 1
                has_l1_late = 1 <= v <= T - 1
                has_l2 = v >= 2
                has_l2_ub = v >= 3
                s = v - 2

                if v + 2 <= T - 1:
                    xt[v + 2] = xpool.tile([D, BL], BF16, tag="x",
                                           name=f"x{v+2}")
                    nc.sync.dma_start(xt[v + 2][:], x_d[v + 2])

                # one psum bank-tile per gate chunk
                ps = {k: pspool.tile([D, 2 * BL], F32, tag=f"ps{k}",
                                     name=f"ps{k}_{v}") for k in CHUNKS}
                g = {k: gpool.tile([D, 2 * BL], BF16, tag=f"g{k}",
                                   name=f"g{k}_{v}") for k in CHUNKS}

                # ---- planned psum writers per chunk, in required order ----
                plan = {k: [] for k in CHUNKS}
                if has_l1:
                    for k in CHUNKS:
                        plan[k].append((ps[k][:, L1], wk(k), xt[v][:]))
                if has_l2:
                    for k in CHUNKS:
                        plan[k].append((ps[k][:, L2], wk(k), h1[s][:]))
                if has_l2_ub:
                    for k in CHUNKS:
                        plan[k].append((ps[k][:, L2], uk(k), h1[s - 1][:]))
                n_early = {k: len(plan[k]) for k in CHUNKS}
                for k in CHUNKS:
                    if has_l1_late:
                        plan[k].append((ps[k][:, L1], uk(k), h1[v - 1][:]))
                    if has_l2_ub:
                        plan[k].append((ps[k][:, L2], uk(k), hr[s - 1][:]))

                nemit = {k: 0 for k in CHUNKS}
                last_mm = {k: None for k in CHUNKS}

                def emit(k):
                    i = nemit[k]
                    out, lhsT, rhs = plan[k][i]
                    m = nc.tensor.matmul(out, lhsT, rhs,
                                         start=(i == 0),
                                         stop=(i == len(plan[k]) - 1))
                    if last_mm[k] is not None:
                        tile.add_dep_helper(m.ins, last_mm[k].ins,
                                            sync=False,
                                            reason="psum bank order")
                    nemit[k] += 1
                    last_mm[k] = m
                    return m

                # earlies, round-robin across chunks (f first)
                earlies = []
                while any(nemit[k] < n_early[k] for k in CHUNKS):
                    for k in CHUNKS:
                        if nemit[k] < n_early[k]:
                            earlies.append(emit(k))
                if prev_last_late is not None:
                    for m in earlies:
                        tile.add_dep_helper(m.ins, prev_last_late.ins,
                                            sync=False,
                                            reason="early after prev lates")

                # ---- lates + activation ladder ----
                def acols(k):
                    if has_l1 and has_l2:
                        return slice(0, 2 * BL)
                    return L1 if has_l1 else L2

                last_late = None
                for k in CHUNKS:
                    while nemit[k] < len(plan[k]):
                        last_late = emit(k)
                    nc.scalar.activation(g[k][:, acols(k)],
                                         ps[k][:, acols(k)],
                                         FUNC[k], bias=bk(k))
                if last_late is not None:
                    prev_last_late = last_late

                # ---- pointwise tails (plain tensor_tensor, DVE 2x) ----
                tcb = tpool.tile([D, 2 * BL], BF16, tag="tc", name=f"tc{v}")

                if has_l1:
                    if has_l1_late:
                        m1a = tpool.tile([D, BL], BF16, tag="m1a",
                                         name=f"m1a{v}")
                        m2a = tpool.tile([D, BL], BF16, tag="m2a",
                                         name=f"m2a{v}")
                        nc.vector.tensor_mul(m1a[:], g[GF][:, L1],
                                             c_both[:, L1])
                        nc.vector.tensor_mul(m2a[:], g[GI][:, L1],
                                             g[GG][:, L1])
                        nc.vector.tensor_add(c_both[:, L1], m1a[:], m2a[:])
                    else:
                        nc.vector.tensor_mul(c_both[:, L1], g[GI][:, L1],
                                             g[GG][:, L1])
                if has_l2:
                    if has_l2_ub:
                        m1b = tpool.tile([D, BL], BF16, tag="m1b",
                                         name=f"m1b{v}")
                        m2b = tpool.tile([D, BL], BF16, tag="m2b",
                                         name=f"m2b{v}")
                        nc.vector.tensor_mul(m1b[:], g[GF][:, L2],
                                             c_both[:, L2])
                        nc.vector.tensor_mul(m2b[:], g[GI][:, L2],
                                             g[GG][:, L2])
                        nc.vector.tensor_add(c_both[:, L2], m1b[:], m2b[:])
                    else:
                        nc.vector.tensor_mul(c_both[:, L2], g[GI][:, L2],
                                             g[GG][:, L2])

                if has_l1:
                    nc.scalar.activation(tcb[:, L1], c_both[:, L1], TANH)
                if has_l2:
                    nc.scalar.activation(tcb[:, L2], c_both[:, L2], TANH)

                if has_l1:
                    nh1 = hpool.tile([D, BL], BF16, tag="h1", name=f"h1_{v}")
                    nc.vector.tensor_mul(nh1[:], g[GO][:, L1], tcb[:, L1])
                    h1[v] = nh1
                if has_l2:
                    nhr = hrpool.tile([D, BL], BF16, tag="hr", name=f"hr{v}")
                    nc.vector.tensor_mul(nhr[:], g[GO][:, L2], tcb[:, L2])
                    hr[s] = nhr
                    yt = ypool.tile([D, BL], BF16, tag="yst", name=f"y{s}")
                    nc.gpsimd.tensor_add(yt[:], h1[s][:], nhr[:])
                    nc.sync.dma_start(y_d[s], yt[:])

                xt.pop(v, None)
                h1.pop(v - 4, None)
                hr.pop(s - 2, None)

    nc.finalize()
    return nc


_CACHED = {}


def _get_nc():
    if "nc" not in _CACHED:
        nc = bacc.Bacc("TRN2", target_bir_lowering=False, debug=False,
                       num_devices=NCORES)
        _CACHED["nc"] = _build(nc)
    return _CACHED["nc"]


def kernel(x, W, U, b, seq_len):
    assert x.shape == (B, T, D)
    nc = _get_nc()

    bf = ml_dtypes.bfloat16
    Wc = np.ascontiguousarray(np.asarray(W, dtype=np.float32).astype(bf))
    Uc = np.ascontiguousarray(np.asarray(U, dtype=np.float32).astype(bf))
    bc = np.ascontiguousarray(
        np.asarray(b, dtype=np.float32).reshape(4, D).T)  # [D, 4]

    in_maps = []
    for c in range(NCORES):
        xc = np.ascontiguousarray(
            np.asarray(x[c * BL:(c + 1) * BL], dtype=np.float32)
            .transpose(1, 2, 0).astype(bf))  # [T, D, BL] bf16
        in_maps.append({"x": xc, "w": Wc, "u": Uc, "bias": bc})

    res = run_bass_kernel_spmd(nc, in_maps, core_ids=list(range(NCORES)))

    y = np.empty((B, T, D), dtype=np.float32)
    for c in range(NCORES):
        y[c * BL:(c + 1) * BL] = (
            res.results[c]["y"].astype(np.float32).transpose(2, 0, 1))
    return y


# revision 10
# speedup vs baseline: 1.1634x; 1.0023x over previous
"""Trainium2 Bass kernel for a 2-layer shared-weight LSTM with residual.

Problem: x:[1024,200,128], W/U:[128,512], b:[512]; two stacked LSTM layers
sharing (W,U,b); layer 2 has a residual connection; seq_len is ignored by the
reference (full T steps).

Sharding: data-parallel over batch: 1024 = 8 cores x 128 rows.

Device layout: gate features on SBUF partitions, batch on the free axis.
Host pre-transposes x to [T, D, B_local] (bf16).

Schedule: layer-2 phase delay d=2 — unit v computes L1 step v and L2 step
v-2.  With the residual split (U*h2n = U*h1 + U*h2raw), every L2 matmul
input except U*h2raw(v-3) is at least two units old, so per gate chunk only
TWO late matmuls (U*h1(v-1) -> L1 half, U*h2raw(v-3) -> L2 half) sit between
h1-ready and that chunk's activation.  Earlies of unit v+1 are dep-pinned
after unit v's LAST late so the scheduler cannot wedge them into the ladder.

psum: one [128,256] tile (= one full psum bank after padding) per gate
chunk, exactly one start=True opener per bank executing first (start=True
clears has_written for the WHOLE bank), stop=True on the final writer.

The pointwise tail is plain tensor_tensor ops (DVE 2x mode, ~136ns chain
pitch): m1=sig(f)*c, m2=sig(i)*tanh(g), c'=m1+m2, tanh(c'), h=sig(o)*tc.
y(s) = h1(s) + h2raw(s) on GpSimd, off the recurrence entirely.
"""

import numpy as np
import ml_dtypes

import concourse.bass as bass
import concourse.tile as tile
from concourse import bacc, mybir
from concourse.bass_utils import run_bass_kernel_spmd

B, T, D = 1024, 200, 128
NCORES = 8
BL = B // NCORES  # 128 batch rows per core
NU = T + 2        # units: 0..T+1 (L1 at v<=T-1, L2 step v-2 at v>=2)

F32 = mybir.dt.float32
BF16 = mybir.dt.bfloat16

# gate order in W/U/b: i, f, g, o  (Keras LSTMCell)
GI, GF, GG, GO = 0, 1, 2, 3
CHUNKS = (GF, GI, GG, GO)  # ladder order: f first (m1 = sig(f)*c earliest)


def _build(nc):
    x_d = nc.dram_tensor("x", [T, D, BL], BF16, kind="ExternalInput")
    w_d = nc.dram_tensor("w", [D, 4 * D], BF16, kind="ExternalInput")
    u_d = nc.dram_tensor("u", [D, 4 * D], BF16, kind="ExternalInput")
    b_d = nc.dram_tensor("bias", [D, 4], F32, kind="ExternalInput")
    y_d = nc.dram_tensor("y", [T, D, BL], BF16, kind="ExternalOutput")

    SIG = mybir.ActivationFunctionType.Sigmoid
    TANH = mybir.ActivationFunctionType.Tanh
    FUNC = {GF: SIG, GI: SIG, GG: TANH, GO: SIG}

    L2 = slice(0, BL)        # L2 half of each chunk (psum + gate tiles)
    L1 = slice(BL, 2 * BL)   # L1 half

    with tile.TileContext(nc) as tc:
        with (
            tc.tile_pool(name="singles", bufs=1) as singles,
            tc.tile_pool(name="xbuf", bufs=4) as xpool,
            tc.tile_pool(name="hbuf", bufs=5) as hpool,
            tc.tile_pool(name="hrbuf", bufs=3) as hrpool,
            tc.tile_pool(name="psum", bufs=2, space="PSUM") as pspool,
            tc.tile_pool(name="gates", bufs=2) as gpool,
            tc.tile_pool(name="tmp", bufs=3) as tpool,
            tc.tile_pool(name="yst", bufs=4) as ypool,
        ):
            w_sb = singles.tile([D, 4 * D], BF16)
            u_sb = singles.tile([D, 4 * D], BF16)
            b_sb = singles.tile([D, 4], F32)
            nc.sync.dma_start(w_sb[:], w_d[:])
            nc.sync.dma_start(u_sb[:], u_d[:])
            nc.sync.dma_start(b_sb[:], b_d[:])

            # persistent cell state: cols 0:BL = c2, BL:2BL = c1 (bf16)
            c_both = singles.tile([D, 2 * BL], BF16)
            nc.vector.memset(c_both[:], 0.0)

            def wk(k):
                return w_sb[:, k * D:(k + 1) * D]

            def uk(k):
                return u_sb[:, k * D:(k + 1) * D]

            def bk(k):
                return b_sb[:, k:k + 1]

            xt = {}   # x(v)
            h1 = {}   # h1(v)
            hr = {}   # h2raw(s)

            for v in (0, 1):
                xt[v] = xpool.tile([D, BL], BF16, tag="x", name=f"x{v}")
                nc.sync.dma_start(xt[v][:], x_d[v])

            prev_last_late = None

            for v in range(NU):
                has_l1 = v <= T - 1
                has_l1_late = 1 <= v <= T - 1
                has_l2 = v >= 2
                has_l2_ub = v >= 3
                s = v - 2

                if v + 2 <= T - 1:
                    xt[v + 2] = xpool.tile([D, BL], BF16, tag="x",
                                           name=f"x{v+2}")
                    nc.sync.dma_start(xt[v + 2][:], x_d[v + 2])

                # one psum bank-tile per gate chunk
                ps = {k: pspool.tile([D, 2 * BL], F32, tag=f"ps{k}",
                                     name=f"ps{k}_{v}") for k in CHUNKS}
                g = {k: gpool.tile([D, 2 * BL], BF16, tag=f"g{k}",
                                   name=f"g{k}_{v}") for k in CHUNKS}

                # ---- planned psum writers per chunk, in required order ----
                plan = {k: [] for k in CHUNKS}
                if has_l1:
                    for k in CHUNKS:
                        plan[k].append((ps[k][:, L1], wk(k), xt[v][:]))
                if has_l2:
                    for k in CHUNKS:
                        plan[k].append((ps[k][:, L2], wk(k), h1[s][:]))
                if has_l2_ub:
                    for k in CHUNKS:
                        plan[k].append((ps[k][:, L2], uk(k), h1[s - 1][:]))
                n_early = {k: len(plan[k]) for k in CHUNKS}
                for k in CHUNKS:
                    if has_l1_late:
                        plan[k].append((ps[k][:, L1], uk(k), h1[v - 1][:]))
                    if has_l2_ub:
                        plan[k].append((ps[k][:, L2], uk(k), hr[s - 1][:]))

                nemit = {k: 0 for k in CHUNKS}
                last_mm = {k: None for k in CHUNKS}

                def emit(k):
                    i = nemit[k]
                    out, lhsT, rhs = plan[k][i]
                    m = nc.tensor.matmul(out, lhsT, rhs,
                                         start=(i == 0),
                                         stop=(i == len(plan[k]) - 1))
                    if last_mm[k] is not None:
                        tile.add_dep_helper(m.ins, last_mm[k].ins,
                                            sync=False,
                                            reason="psum bank order")
                    nemit[k] += 1
                    last_mm[k] = m
                    return m

                # earlies, round-robin across chunks (f first)
                earlies = []
                while any(nemit[k] < n_early[k] for k in CHUNKS):
                    for k in CHUNKS:
                        if nemit[k] < n_early[k]:
                            earlies.append(emit(k))
                if prev_last_late is not None:
                    for m in earlies:
                        tile.add_dep_helper(m.ins, prev_last_late.ins,
                                            sync=False,
                                            reason="early after prev lates")

                # ---- lates + activation ladder ----
                def acols(k):
                    if has_l1 and has_l2:
                        return slice(0, 2 * BL)
                    return L1 if has_l1 else L2

                last_late = None
                first_late_done = False
                for k in CHUNKS:
                    while nemit[k] < len(plan[k]):
                        last_late = emit(k)
                        if not first_late_done:
                            # the unit's first late must come after ALL its
                            # earlies in the linear PE order, else the
                            # scheduler wedges leftover earlies into the
                            # ladder (and idles the PE behind the h1 wait)
                            for m in earlies:
                                tile.add_dep_helper(last_late.ins, m.ins,
                                                    sync=False,
                                                    reason="lates after earlies")
                            first_late_done = True
                    nc.scalar.activation(g[k][:, acols(k)],
                                         ps[k][:, acols(k)],
                                         FUNC[k], bias=bk(k))
                if last_late is not None:
                    prev_last_late = last_late

                # ---- pointwise tails (plain tensor_tensor, DVE 2x) ----
                tcb = tpool.tile([D, 2 * BL], BF16, tag="tc", name=f"tc{v}")

                if has_l1:
                    if has_l1_late:
                        m1a = tpool.tile([D, BL], BF16, tag="m1a",
                                         name=f"m1a{v}")
                        m2a = tpool.tile([D, BL], BF16, tag="m2a",
                                         name=f"m2a{v}")
                        nc.vector.tensor_mul(m1a[:], g[GF][:, L1],
                                             c_both[:, L1])
                        nc.vector.tensor_mul(m2a[:], g[GI][:, L1],
                                             g[GG][:, L1])
                        c1_add = nc.vector.tensor_add(c_both[:, L1],
                                                      m1a[:], m2a[:])
                    else:
                        c1_add = nc.vector.tensor_mul(c_both[:, L1],
                                                      g[GI][:, L1],
                                                      g[GG][:, L1])
                if has_l2:
                    if has_l2_ub:
                        m1b = tpool.tile([D, BL], BF16, tag="m1b",
                                         name=f"m1b{v}")
                        m2b = tpool.tile([D, BL], BF16, tag="m2b",
                                         name=f"m2b{v}")
                        nc.vector.tensor_mul(m1b[:], g[GF][:, L2],
                                             c_both[:, L2])
                        i2 = nc.vector.tensor_mul(m2b[:], g[GI][:, L2],
                                                  g[GG][:, L2])
                        if has_l1_late:
                            # keep the critical c1-add ahead of L2 muls on
                            # the in-order DVE queue
                            tile.add_dep_helper(i2.ins, c1_add.ins,
                                                sync=False,
                                                reason="c1 before L2 tail")
                        nc.vector.tensor_add(c_both[:, L2], m1b[:], m2b[:])
                    else:
                        nc.vector.tensor_mul(c_both[:, L2], g[GI][:, L2],
                                             g[GG][:, L2])

                if has_l1:
                    nc.scalar.activation(tcb[:, L1], c_both[:, L1], TANH)
                if has_l2:
                    nc.scalar.activation(tcb[:, L2], c_both[:, L2], TANH)

                if has_l1:
                    nh1 = hpool.tile([D, BL], BF16, tag="h1", name=f"h1_{v}")
                    nc.vector.tensor_mul(nh1[:], g[GO][:, L1], tcb[:, L1])
                    h1[v] = nh1
                if has_l2:
                    nhr = hrpool.tile([D, BL], BF16, tag="hr", name=f"hr{v}")
                    nc.vector.tensor_mul(nhr[:], g[GO][:, L2], tcb[:, L2])
                    hr[s] = nhr
                    yt = ypool.tile([D, BL], BF16, tag="yst", name=f"y{s}")
                    nc.gpsimd.tensor_add(yt[:], h1[s][:], nhr[:])
                    nc.sync.dma_start(y_d[s], yt[:])

                xt.pop(v, None)
                h1.pop(v - 4, None)
                hr.pop(s - 2, None)

    nc.finalize()
    return nc


_CACHED = {}


def _get_nc():
    if "nc" not in _CACHED:
        nc = bacc.Bacc("TRN2", target_bir_lowering=False, debug=False,
                       num_devices=NCORES)
        _CACHED["nc"] = _build(nc)
    return _CACHED["nc"]


def kernel(x, W, U, b, seq_len):
    assert x.shape == (B, T, D)
    nc = _get_nc()

    bf = ml_dtypes.bfloat16
    Wc = np.ascontiguousarray(np.asarray(W, dtype=np.float32).astype(bf))
    Uc = np.ascontiguousarray(np.asarray(U, dtype=np.float32).astype(bf))
    bc = np.ascontiguousarray(
        np.asarray(b, dtype=np.float32).reshape(4, D).T)  # [D, 4]

    in_maps = []
    for c in range(NCORES):
        xc = np.ascontiguousarray(
            np.asarray(x[c * BL:(c + 1) * BL], dtype=np.float32)
            .transpose(1, 2, 0).astype(bf))  # [T, D, BL] bf16
        in_maps.append({"x": xc, "w": Wc, "u": Uc, "bias": bc})

    res = run_bass_kernel_spmd(nc, in_maps, core_ids=list(range(NCORES)))

    y = np.empty((B, T, D), dtype=np.float32)
    for c in range(NCORES):
        y[c * BL:(c + 1) * BL] = (
            res.results[c]["y"].astype(np.float32).transpose(2, 0, 1))
    return y
